# revision 59
# baseline (speedup 1.0000x reference)
"""Segmented factored-form BCJR detector for Trainium2.

Algorithm (validated in proto2.py): run the sparse 2-term trellis
recursions directly, data-parallel over K time segments per direction
(sliding-window BCJR, W warmup steps from a uniform start; the padded
g columns [1,0,...] pin the exact delta_0 boundary condition).

State and g live in a bit-rotated column order pi(s) = 8*(s&1)+(s>>1)
so that every chain multiply hits the DVE packed-innermost 2x mode:

  alpha:  s2[q] = A[q] + A[q+8]           (DVE, 1x: permuted out)
          A'[2q+b] = s2[q] * g[2q+b]      (DVE, 2x in pi layout)
  beta:   t2[r] = B[2r] + B[2r+1]         (Pool; pi makes it halves)
          B'[8h+r] = t2[r] * g[8h+r]      (DVE, 2x in pi layout)
  decode: w = g.*B' ; dq[q] = w[2q]-w[2q+1]   (Pool, inline, deferred)
          D_t = sum_q s2a_t[q]*dq_t[q] ; bit = D_t < 0   (batched tail)

g[t,s] = exp(scale*(y_t-sp_s)^2 + bias): Pool(sub) + Act(Square, Exp),
generated in two opposing sweeps + a fine-grained head that exactly
tile the padded time axis just ahead of chain consumption.
"""

import math
import sys

import numpy as np

sys.path.insert(0, "/opt/trn_rl_repo")

B, T, S, MEM, V = 1024, 2048, 16, 4, 4
NCORES = 8
BPC = B // NCORES

K = 32            # time segments per direction
L = T // K        # segment length
W = 12            # warmup steps
P = L + W         # positions per segment chain
NP = 48           # normalize every NP positions
GC = 8            # g-gen sweep chunk (positions)
HC = 4            # g-gen head chunk (columns)
TPAD = T + 2 * W  # padded time axis

PI = [8 * (s & 1) + (s >> 1) for s in range(S)]   # storage col of state s


def _mkap(win, dims):
    """Custom strided view: win is a [:, a:b] AP window giving base offset."""
    import bass_rust
    v = win.copy()
    part = list(v.ap)[0]
    v.ap = bass_rust.VecI64Pair([list(part)] + [list(d) for d in dims])
    return v


def _build(nc, cfg):
    import concourse.bass as bass  # noqa: F401
    from concourse import mybir, tile
    from concourse.alu_op_type import AluOpType as OP
    from concourse.mybir import ActivationFunctionType as AF

    f32 = mybir.dt.float32
    bf16 = mybir.dt.bfloat16
    scale, bias = float(cfg["scale"]), float(cfg["bias"])

    # yin: [W zeros | y (T) | W zeros | sp_pi (S)], bf16
    yin_d = nc.dram_tensor("yin", [BPC, TPAD + S], bf16, kind="ExternalInput")
    out_d = nc.dram_tensor("dec", [BPC, T], f32, kind="ExternalOutput")

    ctx = nc.allow_low_precision(reason="bf16 chains; decisions are sign/tie based")
    ctx.__enter__()
    with tile.TileContext(nc) as tc_:
        with (
            tc_.tile_pool(name="slab", bufs=1) as slab,
            tc_.tile_pool(name="gring", bufs=2) as gring,
            tc_.tile_pool(name="st", bufs=1) as st,
            tc_.tile_pool(name="cmb", bufs=1) as cmb,
        ):
            G = slab.tile([BPC, 16 * TPAD], bf16, name="G", tag="G")
            S2A = slab.tile([BPC, 8 * T + 8 * K], bf16, name="S2A", tag="S2A")
            DQ = slab.tile([BPC, 8 * T], bf16, name="DQ", tag="DQ")
            ysb = slab.tile([BPC, TPAD], bf16, name="ysb", tag="ysb")
            spt = st.tile([BPC, S], bf16, name="spt", tag="spt")
            Ast = st.tile([BPC, 16 * K], bf16, name="Ast", tag="Ast")
            Bst = [st.tile([BPC, 16 * K], bf16, name=f"Bst{i}", tag=f"Bst{i}")
                   for i in range(2)]
            T2 = st.tile([BPC, 8 * K], bf16, name="T2", tag="T2")
            Wt = st.tile([BPC, 16 * K], bf16, name="Wt", tag="Wt")
            asum = st.tile([BPC, K], f32, name="asum", tag="asum")
            bsum = st.tile([BPC, K], f32, name="bsum", tag="bsum")
            arec = st.tile([BPC, K], f32, name="arec", tag="arec")
            brec = st.tile([BPC, K], f32, name="brec", tag="brec")
            bias_t = st.tile([BPC, 1], f32, name="bias_t", tag="bias_t")
            zero_t = st.tile([BPC, 1], f32, name="zero_t", tag="zero_t")

            nc.vector.memset(bias_t[:, :], bias)
            nc.vector.memset(zero_t[:, :], 0.0)
            nc.sync.dma_start(ysb[:, :], yin_d[:, 0:TPAD])
            nc.sync.dma_start(spt[:, :], yin_d[:, TPAD:TPAD + S])

            def gen_g(c0, cw, nblk, sub_eng, sq_eng=None):
                """g cols {i*L + c : i in [0,nblk), c in [c0,c0+cw)}."""
                d = gring.tile([BPC, nblk * cw * 16], bf16,
                               name=f"d_{c0}_{cw}", tag=f"dg{cw}")
                d2 = gring.tile([BPC, nblk * cw * 16], bf16,
                                name=f"d2_{c0}_{cw}", tag=f"d2g{cw}")
                dv = _mkap(d[:, 0:1], [(16 * cw, nblk), (16, cw), (1, 16)])
                yv = _mkap(ysb[:, c0:c0 + 1], [(L, nblk), (1, cw), (0, 16)])
                spv = _mkap(spt[:, 0:1], [(0, nblk), (0, cw), (1, 16)])
                sub_eng.tensor_tensor(dv, yv, spv, OP.subtract)
                if sq_eng is None:
                    nc.scalar.activation(d2[:, :], d[:, :], AF.Square,
                                         bias=zero_t[:, :], scale=1.0)
                else:
                    sq_eng.tensor_tensor(d2[:, :], d[:, :], d[:, :], OP.mult)
                gv = _mkap(G[:, 16 * c0:16 * c0 + 1],
                           [(16 * L, nblk), (16, cw), (1, 16)])
                d2v = _mkap(d2[:, 0:1], [(16 * cw, nblk), (16, cw), (1, 16)])
                nc.scalar.activation(gv, d2v, AF.Exp,
                                     bias=bias_t[:, :], scale=scale)

            def pads():
                # pattern [1,0,...,0] per pad column (pi(0)=0)
                nc.gpsimd.memset(G[:, 0:16 * W], 0.0)
                nc.gpsimd.memset(_mkap(G[:, 0:1], [(16, W)]), 1.0)
                b0 = 16 * (T + W)
                nc.gpsimd.memset(G[:, b0:b0 + 16 * W], 0.0)
                nc.gpsimd.memset(_mkap(G[:, b0:b0 + 1], [(16, W)]), 1.0)

            def chain_pos(p, wdq_of):
                ga = 16 * p                     # alpha g base (tau = i*L+p)
                gb = 16 * (L + 2 * W - 1 - p)   # beta g base
                bcur, bprev = Bst[p % 2], Bst[1 - p % 2]
                # --- beta add first: shortens the cross-engine round trip ---
                t2v = _mkap(T2[:, 0:1], [(8, K), (4, 2), (1, 4)])
                b0v = _mkap(bprev[:, 0:1], [(16, K), (1, 2), (2, 4)])
                b1v = _mkap(bprev[:, 8:9], [(16, K), (1, 2), (2, 4)])
                nc.gpsimd.tensor_tensor(t2v, b0v, b1v, OP.add)
                # --- alpha: s2 (natural order) -> slab ----------------------
                if p >= W:
                    sb = 8 * (p - W)
                    sdim = [(8 * L, K)]
                else:
                    sb = 8 * T
                    sdim = [(8, K)]
                s2v = _mkap(S2A[:, sb:sb + 1], sdim + [(1, 2), (2, 4)])
                a0 = _mkap(Ast[:, 0:1], [(16, K), (8, 2), (1, 4)])
                a1 = _mkap(Ast[:, 4:5], [(16, K), (8, 2), (1, 4)])
                nc.vector.tensor_tensor(s2v, a0, a1, OP.add)
                # A'[pi(2q+b)=8b+q] = s2[q]*g[2q+b]  (all packed: 2x)
                av = _mkap(Ast[:, 0:1], [(16, K), (8, 2), (1, 8)])
                s2m = _mkap(S2A[:, sb:sb + 1], sdim + [(0, 2), (1, 8)])
                gav = _mkap(G[:, ga:ga + 1], [(16 * L, K), (8, 2), (1, 8)])
                nc.vector.tensor_tensor(av, s2m, gav, OP.mult)
                # --- beta mult: B'[pi(8h+r)=8r0+4h+rh] = t2[r]*g[8h+r] ------
                bv = _mkap(bcur[:, 0:1], [(16, K), (8, 2), (4, 2), (1, 4)])
                t2m = _mkap(T2[:, 0:1], [(8, K), (4, 2), (0, 2), (1, 4)])
                gbv = _mkap(G[:, gb:gb + 1], [(16 * L, K), (8, 2), (4, 2), (1, 4)])
                nc.vector.tensor_tensor(bv, t2m, gbv, OP.mult)
                # --- deferred w/dq for an earlier position ------------------
                if wdq_of is not None:
                    q = wdq_of
                    bq = Bst[q % 2]
                    gq = 16 * (L + 2 * W - 1 - q)
                    wv = _mkap(Wt[:, 0:1], [(16, K), (1, 16)])
                    gwv = _mkap(G[:, gq:gq + 1], [(16 * L, K), (1, 16)])
                    bwv = _mkap(bq[:, 0:1], [(16, K), (1, 16)])
                    nc.gpsimd.tensor_tensor(wv, gwv, bwv, OP.mult)
                    dqb = 8 * (L + W - 1 - q)
                    dqv = _mkap(DQ[:, dqb:dqb + 1],
                                [(8 * L, K), (1, 2), (2, 4)])
                    w0 = _mkap(Wt[:, 0:1], [(16, K), (1, 2), (2, 4)])
                    w1 = _mkap(Wt[:, 8:9], [(16, K), (1, 2), (2, 4)])
                    nc.gpsimd.tensor_tensor(dqv, w0, w1, OP.subtract)

            def norm():
                a3 = Ast[:, :].rearrange("p (k s) -> p k s", s=16)
                nc.vector.tensor_reduce(asum[:, :], a3, mybir.AxisListType.X,
                                        OP.add)
                nc.vector.reciprocal(arec[:, :], asum[:, :])
                rv = _mkap(arec[:, 0:1], [(1, K), (0, 16)])
                avo = _mkap(Ast[:, 0:1], [(16, K), (1, 16)])
                avi = _mkap(Ast[:, 0:1], [(16, K), (1, 16)])
                nc.vector.tensor_tensor(avo, avi, rv, OP.mult)

            def norm_beta(p):
                bcur = Bst[p % 2]
                b3 = bcur[:, :].rearrange("p (k s) -> p k s", s=16)
                nc.vector.tensor_reduce(bsum[:, :], b3, mybir.AxisListType.X,
                                        OP.add)
                nc.vector.reciprocal(brec[:, :], bsum[:, :])
                rbv = _mkap(brec[:, 0:1], [(1, K), (0, 16)])
                bvo = _mkap(bcur[:, 0:1], [(16, K), (1, 16)])
                bvi = _mkap(bcur[:, 0:1], [(16, K), (1, 16)])
                nc.gpsimd.tensor_tensor(bvo, bvi, rbv, OP.mult)

            def combine(o0, no, eng=None):
                """Decode t = i*L + o for o in [o0,o0+no), all K segments.

                The whole E/tree chain runs on ONE engine so a combine never
                serializes the other engine's stream."""
                ce = eng or nc.vector
                n = K * no
                E = cmb.tile([BPC, 8 * n], bf16, name=f"E_{o0}", tag="E")
                e4 = cmb.tile([BPC, 4 * n], bf16, name=f"e4_{o0}", tag="e4")
                e2 = cmb.tile([BPC, 2 * n], bf16, name=f"e2_{o0}", tag="e2")
                Dt = cmb.tile([BPC, n], bf16, name=f"Dt_{o0}", tag="Dt")
                dect = cmb.tile([BPC, n], f32, name=f"dec_{o0}", tag="dect")
                sv = _mkap(S2A[:, 8 * o0:8 * o0 + 1], [(8 * L, K), (1, 8 * no)])
                dv = _mkap(DQ[:, 8 * o0:8 * o0 + 1], [(8 * L, K), (1, 8 * no)])
                ce.tensor_tensor(E[:, :], sv, dv, OP.mult)
                i0 = _mkap(E[:, 0:1], [(8, n), (1, 4)])
                i1 = _mkap(E[:, 4:5], [(8, n), (1, 4)])
                ce.tensor_tensor(e4[:, :], i0, i1, OP.add)
                j0 = _mkap(e4[:, 0:1], [(4, n), (1, 2)])
                j1 = _mkap(e4[:, 2:3], [(4, n), (1, 2)])
                ce.tensor_tensor(e2[:, :], j0, j1, OP.add)
                k0 = _mkap(e2[:, 0:1], [(2, n)])
                k1 = _mkap(e2[:, 1:2], [(2, n)])
                ce.tensor_tensor(Dt[:, :], k0, k1, OP.add)
                nc.gpsimd.tensor_scalar(dect[:, :], Dt[:, :], 0.0, None,
                                        OP.is_lt)
                ov = _mkap(out_d[:, o0:o0 + 1], [(L, K), (1, no)])
                nc.sync.dma_start(ov, dect[:, :])

            # ================= schedule ==================================
            nc.vector.memset(Ast[:, :], 1.0)
            nc.vector.memset(Bst[0][:, :], 1.0)
            nc.vector.memset(Bst[1][:, :], 1.0)
            pads()   # pad cols are memset-only; gen chunks skip them
            # preload the Square/Exp act table while the y DMA runs
            nc.scalar.activation(zero_t[:, :], zero_t[:, :], AF.Square,
                                 bias=zero_t[:, :], scale=1.0)
            # g chunks, enqueued upfront in consumer-need order; Act runs
            # them in order while the bufs=2 rings pipeline sub/Square/Exp.
            # head-low  c in [0,W):   blocks 1..K   (block0 c<W = left pad)
            # head-high c in [W,2W):  blocks 0..K-1 (blockK c>=W = right pad)
            # alpha needs col c at p=c; beta needs col 2W-1-p (next block).
            for j, c0 in enumerate(range(0, W, HC)):
                chi = 2 * W - HC - c0
                gen_g(L + c0, HC, K, nc.vector)           # head-low, shifted
                gen_g(chi if chi >= W else L + chi, HC, K, nc.gpsimd)
            # interior sweeps [2W, L), interleaved desc/asc by need:
            # asc col c needed at p=c, desc col c at p = L+2W-1-c
            mid = (L + 2 * W) // 2
            lo, hi = 2 * W, L
            sflip = 0
            while lo < mid:
                cw = min(GC, mid - lo)
                gen_g(hi - cw, cw, K,
                      nc.vector if sflip % 2 == 0 else nc.gpsimd)
                gen_g(lo, cw, K,
                      nc.vector if sflip % 2 == 1 else nc.gpsimd)
                sflip += 1
                lo += cw
                hi -= cw
            # combine [o0,o0+no): needs s2a (p >= W+o0+no) and dq, whose
            # w/dq op is deferred to position (P-1-o0)+1
            NO = L // 8
            cm_sched = {}
            cm_tail = []
            for o0 in range(0, L, NO):
                pr = max(W + o0 + NO, P - o0)
                while pr in cm_sched:
                    pr += 1
                if pr < P:
                    cm_sched[pr] = (o0, NO)
                else:
                    cm_tail.append((o0, NO))
            cmbflip = [0]
            for p in range(P):
                wdq = p - 1 if (p - 1 >= W) else None
                chain_pos(p, wdq)
                if (p + 1) % NP == 0 and p + 1 < P:
                    norm()
                    norm_beta(p)
                if p in cm_sched:
                    cmbflip[0] += 1
                    combine(*cm_sched[p],
                            eng=nc.gpsimd if cmbflip[0] % 2 else nc.vector)
            # final position's deferred w/dq
            q = P - 1
            bq = Bst[q % 2]
            gq = 16 * (L + 2 * W - 1 - q)
            wv = _mkap(Wt[:, 0:1], [(16, K), (1, 16)])
            gwv = _mkap(G[:, gq:gq + 1], [(16 * L, K), (1, 16)])
            bwv = _mkap(bq[:, 0:1], [(16, K), (1, 16)])
            nc.gpsimd.tensor_tensor(wv, gwv, bwv, OP.mult)
            dqb = 8 * (L + W - 1 - q)
            dqv = _mkap(DQ[:, dqb:dqb + 1], [(8 * L, K), (1, 2), (2, 4)])
            w0 = _mkap(Wt[:, 0:1], [(16, K), (1, 2), (2, 4)])
            w1 = _mkap(Wt[:, 8:9], [(16, K), (1, 2), (2, 4)])
            nc.gpsimd.tensor_tensor(dqv, w0, w1, OP.subtract)
            for i, (o0, no) in enumerate(cm_tail):
                combine(o0, no, eng=nc.gpsimd if i % 2 else None)
    ctx.__exit__(None, None, None)
    return nc


def _legalize_multiwait(bir):
    """Split multi-sem-wait engine instructions (walrus allows only one)."""
    n = 0
    for fn in bir["functions"]:
        for blk in fn["blocks"]:
            newl = []
            for inst in blk["instructions"]:
                si = inst.get("sync_info") or {}
                waits = si.get("on_wait") or []
                eng = inst.get("engine")
                if len(waits) >= 2 and eng in (
                    "DVE", "Pool", "Activation", "PE", "SP",
                ):
                    for j, w in enumerate(waits):
                        carrier = {
                            "name": inst["name"] + f"-wc{j}",
                            "opcode": "EventSemaphore",
                            "engine": eng,
                            "ins": [],
                            "outs": [],
                            "sync_info": {"on_wait": [w], "on_update": []},
                        }
                        if "debug" in inst:
                            carrier["debug"] = inst["debug"]
                        newl.append(carrier)
                        n += 1
                    si["on_wait"] = []
                    inst["sync_info"] = si
                newl.append(inst)
            blk["instructions"] = newl
    return n


def _finalize(nc):
    import json as _json

    bir = _json.loads(nc.to_json_bytes())
    _legalize_multiwait(bir)
    bts = _json.dumps(bir).encode()
    nc.to_json_bytes = lambda: bts
    return nc


def _prep_inputs(y, h, snr):
    """Host-side: yin rows [W zeros | y | W zeros | sp(pi-permuted)]."""
    sigma = np.float32(10.0 ** (-float(snr) / 10.0))
    bits = (np.arange(S)[:, None] >> np.arange(MEM - 1, -1, -1)) & 1
    syms = (1 - 2 * bits).astype(np.float32)
    sp = (syms @ h[:, ::-1].T).astype(np.float32)        # [S, V]
    scale = np.float32(-1.0 / (2.0 * sigma * sigma))
    bias = np.float32(-math.log(math.sqrt(2.0 * math.pi) * sigma))
    Bn = y.shape[0]
    sp_full = sp.T[np.arange(Bn) % V]                    # [Bn, S]
    sp_pi = np.empty_like(sp_full)
    for s in range(S):
        sp_pi[:, PI[s]] = sp_full[:, s]
    return sp_pi, scale, bias


def kernel(y, h, snr):
    import concourse.bass as bass
    from concourse.bass_utils import run_bass_kernel_spmd

    y = np.ascontiguousarray(np.asarray(y, dtype=np.float32))
    h = np.ascontiguousarray(np.asarray(h, dtype=np.float32))
    snr_f = float(np.asarray(snr))
    sp_pi, scale, bias = _prep_inputs(y, h, snr_f)

    nc = bass.Bass()
    _build(nc, dict(scale=scale, bias=bias))
    _finalize(nc)

    from ml_dtypes import bfloat16
    zw = np.zeros((BPC, W), np.float32)
    in_maps = []
    for c in range(NCORES):
        rows = slice(c * BPC, (c + 1) * BPC)
        yin = np.concatenate([zw, y[rows], zw, sp_pi[rows]], axis=1)
        in_maps.append({"yin": np.ascontiguousarray(yin.astype(bfloat16))})
    res = run_bass_kernel_spmd(nc, in_maps, core_ids=list(range(NCORES)))
    dec = np.concatenate([r["dec"] for r in res.results], axis=0)  # [B, T]

    out = np.zeros((B, T), np.float32)
    out[:, MEM - 1:] = dec[:, :T - (MEM - 1)]
    return out
